# revision 14
# baseline (speedup 1.0000x reference)
"""Trainium2 Bass kernel for nn_ExampleTiedDropout (scatter_memory).

reference: out = X * mask[:, :, None] where mask[b] has the first
int(0.2*S)=204 positions fixed to 1 and the remaining 820 positions
Bernoulli(0.1) keyed by fold_in(key(0), idx[b]).

Since the mask is exactly {0, 1}, the output is a row-sparse copy of X:
~28% of the (b, s) rows are copied verbatim, the rest are zero. The
kernel therefore:
  1. computes the mask on host with the same jax ops as the reference
     (bit-exact: same env -> same rbg backend bitstream),
  2. shards the batch data-parallel across 8 NeuronCores (4 examples,
     i.e. 4096 rows of 2048 floats, per core),
  3. on each core, gathers the kept rows HBM->SBUF and scatters them
     back to the output with indirect DMAs driven by a host-built row
     index table (padded entries are out-of-bounds and skipped),
  4. leaves dropped rows untouched -- ExternalOutput buffers are
     donated zero-filled buffers, so unwritten rows read back as 0.
This moves only ~2 x 9.4 MB per core instead of 2 x 33.5 MB dense.
"""
import numpy as np

B, S, H = 32, 1024, 2048
N_CORES = 8
BPC = B // N_CORES           # examples per core
ROWS = BPC * S               # rows of H floats per core
P = 128                      # SBUF partitions
P_FIXED, P_MEM, MASK_SEED = 0.2, 0.1, 0
OOB_IDX = 1 << 24            # padded index; > bounds_check -> DMA skips it

_PROGRAM_CACHE = {}
LAST_RESULTS = None


def _ensure_ntff_hook():
    """The concourse trace path imports antenv.axon_hooks, which this image's
    antenv package lacks -- a hard crash when tracing is requested. Provide
    the missing module and register the boot's ctypes-based hook so NTFF
    profiling works as designed. No-op when the real module exists."""
    try:
        import antenv.axon_hooks  # noqa: F401
        return
    except ImportError:
        pass
    import sys
    import types

    mod = types.ModuleType("antenv.axon_hooks")
    mod._hook = None
    mod.set_axon_ntff_profile_hook = lambda h: setattr(mod, "_hook", h)
    mod.get_axon_ntff_profile_hook = lambda: mod._hook
    sys.modules["antenv.axon_hooks"] = mod
    try:
        import antenv
        antenv.axon_hooks = mod
    except ImportError:
        pass
    try:
        from trn_agent_boot.trn_boot import _ntff_profile_via_ctypes
        mod._hook = _ntff_profile_via_ctypes("/opt/axon/libaxon_pjrt.so")
    except Exception:
        pass  # hook stays None: concourse logs a warning and skips tracing


_ensure_ntff_hook()


def _tied_dropout_mask_host(idx_np):
    """Verbatim replica of reference._tied_dropout_mask, evaluated with the
    process-default jax backend/PRNG so the bits match the grader's
    reference run in the same environment."""
    import jax
    import jax.numpy as jnp

    n_fixed = int(P_FIXED * S)
    n_rand = S - n_fixed
    base = jax.random.key(MASK_SEED)

    def row_mask(i):
        k = jax.random.fold_in(base, i)
        return jax.random.bernoulli(k, P_MEM, (n_rand,)).astype(jnp.float32)

    idx = jnp.asarray(idx_np)
    rand_part = jax.vmap(row_mask)(idx)
    fixed_part = jnp.ones((idx.shape[0], n_fixed), jnp.float32)
    return np.asarray(jnp.concatenate([fixed_part, rand_part], axis=1))


N_FIXED = int(P_FIXED * S)   # 204 leading rows per example, always kept


def _build_program(n_tiles):
    """Static SPMD program per core:
      - one direct DRAM->DRAM HWDGE copy of the fixed prefix rows
        (x[e, :204, :] -> y[e, :204, :] for the 4 local examples),
      - n_tiles x (indirect gather 128 scattered rows -> SBUF,
        indirect scatter back to y) driven by the kidx input.
    Value-agnostic: row choices live in the kidx tensor; padded entries
    are out-of-bounds and skipped by the DMA engine."""
    import concourse.bacc as bacc
    import concourse.bass as bass
    import concourse.tile as tile
    from concourse import mybir

    nc = bacc.Bacc("TRN2", target_bir_lowering=False, debug=False,
                   num_devices=N_CORES, num_swdge_queues=2)
    x = nc.dram_tensor("x", [ROWS, H], mybir.dt.float32, kind="ExternalInput")
    kidx = nc.dram_tensor("kidx", [P, max(n_tiles, 1)], mybir.dt.int32,
                          kind="ExternalInput")
    gidx = nc.dram_tensor("gidx", [P, max(n_tiles, 1) * (P // 16)],
                          mybir.dt.int16, kind="ExternalInput")
    y = nc.dram_tensor("y", [ROWS, H], mybir.dt.float32, kind="ExternalOutput")

    # The fixed prefix of example e (rows [e*S, e*S + N_FIXED)) is one
    # contiguous block of N_FIXED*H floats; view it as [128, N_FIXED*H/128]
    # so the copy is partition-mapped and spreads over all 16 SDMA engines.
    # (A direct DRAM->DRAM copy measures ~100 GB/s: no partition split.)
    FW = N_FIXED * H // P  # 3264

    def fixed_view(ap, e):
        blk = ap[e * S:e * S + N_FIXED, :]          # [204, 2048] contiguous
        return blk.rearrange("s h -> (s h)").rearrange("(p f) -> p f", p=P)

    with tile.TileContext(nc) as tc:
        with (
            tc.tile_pool(name="xp", bufs=max(n_tiles, 1)) as xp,
            tc.tile_pool(name="fp", bufs=BPC) as fp,
            tc.tile_pool(name="ip", bufs=1) as ip,
        ):
            # Fixed prefix: contiguous bulk copy through SBUF. Loads on the
            # SP HWDGE ring, stores on the ACT ring -- one-directional each,
            # both concurrent with the SWDGE indirect stream below.
            for e in range(BPC):
                ft = fp.tile([P, FW], mybir.dt.float32)
                nc.sync.dma_start(out=ft[:], in_=fixed_view(x, e))
                nc.scalar.dma_start(out=fixed_view(y, e), in_=ft[:])

            if n_tiles > 0:
                # Small index loads. The int16 gather table (16 descriptors)
                # loads in ~1us; the int32 scatter table (128 descriptors)
                # takes ~10us of HWDGE descriptor-gen but is only needed by
                # the scatters, which start later anyway.
                it16 = ip.tile([P, n_tiles * (P // 16)], mybir.dt.int16)
                nc.scalar.dma_start(out=it16[:], in_=gidx[:])
                it = ip.tile([P, n_tiles], mybir.dt.int32)
                nc.scalar.dma_start(out=it[:], in_=kidx[:])
                W = P // 16
                for t in range(n_tiles):
                    xt = xp.tile([P, H], mybir.dt.float32)
                    # Gather 128 scattered rows on SWDGE queue 1; scatter
                    # them back on queue 0 -- two concurrent DMA queues.
                    nc.gpsimd.dma_gather(
                        out_ap=xt[:].rearrange("p (a h) -> p a h", a=1),
                        in_ap=x[:],
                        idxs_ap=it16[:, t * W:(t + 1) * W],
                        num_idxs=P,
                        num_idxs_reg=P,
                        elem_size=H,
                        queue_num=1,
                    )
                    nc.gpsimd.indirect_dma_start(
                        out=y[:],
                        out_offset=bass.IndirectOffsetOnAxis(
                            ap=it[:, t:t + 1], axis=0),
                        in_=xt[:],
                        in_offset=None,
                        bounds_check=ROWS - 1,
                        oob_is_err=False,
                    )
    nc.compile()
    return nc


def kernel(X, idx):
    global LAST_RESULTS
    from concourse.bass_utils import run_bass_kernel_spmd

    X = np.ascontiguousarray(np.asarray(X, dtype=np.float32))
    idx = np.asarray(idx, dtype=np.int32)

    mask = _tied_dropout_mask_host(idx)          # [B, S] float32 of {0,1}
    keep = mask.reshape(N_CORES, ROWS) > 0.5     # [8, 4096] bool
    # The fixed prefix rows (s < N_FIXED of each example) are copied by the
    # static bulk DMA; only scattered kept rows go through the index table.
    keep[:, :] &= np.tile(np.arange(S) >= N_FIXED, BPC)[None, :]

    keep_rows = [np.flatnonzero(keep[c]).astype(np.int32) for c in range(N_CORES)]
    max_keep = max(len(r) for r in keep_rows)
    n_tiles = -(-max_keep // P)                  # same static tile count per core

    in_maps = []
    for c in range(N_CORES):
        nt = max(n_tiles, 1)
        r = keep_rows[c]
        # scatter table: int32, OOB padding (skipped); tile t = column t
        flat = np.full((nt * P,), OOB_IDX, dtype=np.int32)
        flat[: len(r)] = r
        kidx = np.ascontiguousarray(flat.reshape(nt, P).T)
        # gather table: int16, slot k of tile t at [k%16, t*8 + k//16],
        # replicated down the partitions for the 8 Q7 cores. Padded with a
        # duplicate valid row (num_idxs_reg must equal the valid count);
        # the scatter's OOB padding discards the duplicates.
        gflat = np.full((nt * P,), r[-1] if len(r) else 0, dtype=np.int16)
        gflat[: len(r)] = r.astype(np.int16)
        blk = np.ascontiguousarray(gflat.reshape(nt * P // 16, 16).T)
        gidx = np.tile(blk, (P // 16, 1))
        in_maps.append({
            "x": X[c * BPC:(c + 1) * BPC].reshape(ROWS, H),
            "kidx": kidx,
            "gidx": gidx,
        })

    if n_tiles not in _PROGRAM_CACHE:
        _PROGRAM_CACHE[n_tiles] = _build_program(n_tiles)
    nc = _PROGRAM_CACHE[n_tiles]

    res = run_bass_kernel_spmd(nc, in_maps, list(range(N_CORES)))
    LAST_RESULTS = res

    out = np.empty((B, S, H), dtype=np.float32)
    for c in range(N_CORES):
        out[c * BPC:(c + 1) * BPC] = res.results[c]["y"].reshape(BPC, S, H)
    return out


# revision 15
# speedup vs baseline: 1.0673x; 1.0673x over previous
"""Trainium2 Bass kernel for nn_ExampleTiedDropout (scatter_memory).

reference: out = X * mask[:, :, None] where mask[b] has the first
int(0.2*S)=204 positions fixed to 1 and the remaining 820 positions
Bernoulli(0.1) keyed by fold_in(key(0), idx[b]).

Since the mask is exactly {0, 1}, the output is a row-sparse copy of X:
~28% of the (b, s) rows are copied verbatim, the rest are zero. The
kernel therefore:
  1. computes the mask on host with the same jax ops as the reference
     (bit-exact: same env -> same rbg backend bitstream),
  2. shards the batch data-parallel across 8 NeuronCores (4 examples,
     i.e. 4096 rows of 2048 floats, per core),
  3. on each core, gathers the kept rows HBM->SBUF and scatters them
     back to the output with indirect DMAs driven by a host-built row
     index table (padded entries are out-of-bounds and skipped),
  4. leaves dropped rows untouched -- ExternalOutput buffers are
     donated zero-filled buffers, so unwritten rows read back as 0.
This moves only ~2 x 9.4 MB per core instead of 2 x 33.5 MB dense.
"""
import numpy as np

B, S, H = 32, 1024, 2048
N_CORES = 8
BPC = B // N_CORES           # examples per core
ROWS = BPC * S               # rows of H floats per core
P = 128                      # SBUF partitions
P_FIXED, P_MEM, MASK_SEED = 0.2, 0.1, 0
OOB_IDX = 1 << 24            # padded index; > bounds_check -> DMA skips it

_PROGRAM_CACHE = {}
LAST_RESULTS = None


def _ensure_ntff_hook():
    """The concourse trace path imports antenv.axon_hooks, which this image's
    antenv package lacks -- a hard crash when tracing is requested. Provide
    the missing module and register the boot's ctypes-based hook so NTFF
    profiling works as designed. No-op when the real module exists."""
    try:
        import antenv.axon_hooks  # noqa: F401
        return
    except ImportError:
        pass
    import sys
    import types

    mod = types.ModuleType("antenv.axon_hooks")
    mod._hook = None
    mod.set_axon_ntff_profile_hook = lambda h: setattr(mod, "_hook", h)
    mod.get_axon_ntff_profile_hook = lambda: mod._hook
    sys.modules["antenv.axon_hooks"] = mod
    try:
        import antenv
        antenv.axon_hooks = mod
    except ImportError:
        pass
    try:
        from trn_agent_boot.trn_boot import _ntff_profile_via_ctypes
        mod._hook = _ntff_profile_via_ctypes("/opt/axon/libaxon_pjrt.so")
    except Exception:
        pass  # hook stays None: concourse logs a warning and skips tracing


_ensure_ntff_hook()


def _tied_dropout_mask_host(idx_np):
    """Verbatim replica of reference._tied_dropout_mask, evaluated with the
    process-default jax backend/PRNG so the bits match the grader's
    reference run in the same environment."""
    import jax
    import jax.numpy as jnp

    n_fixed = int(P_FIXED * S)
    n_rand = S - n_fixed
    base = jax.random.key(MASK_SEED)

    def row_mask(i):
        k = jax.random.fold_in(base, i)
        return jax.random.bernoulli(k, P_MEM, (n_rand,)).astype(jnp.float32)

    idx = jnp.asarray(idx_np)
    rand_part = jax.vmap(row_mask)(idx)
    fixed_part = jnp.ones((idx.shape[0], n_fixed), jnp.float32)
    return np.asarray(jnp.concatenate([fixed_part, rand_part], axis=1))


N_FIXED = int(P_FIXED * S)   # 204 leading rows per example, always kept


def _build_program(n_tiles):
    """Static SPMD program per core:
      - one direct DRAM->DRAM HWDGE copy of the fixed prefix rows
        (x[e, :204, :] -> y[e, :204, :] for the 4 local examples),
      - n_tiles x (indirect gather 128 scattered rows -> SBUF,
        indirect scatter back to y) driven by the kidx input.
    Value-agnostic: row choices live in the kidx tensor; padded entries
    are out-of-bounds and skipped by the DMA engine."""
    import concourse.bacc as bacc
    import concourse.bass as bass
    import concourse.tile as tile
    from concourse import mybir

    nc = bacc.Bacc("TRN2", target_bir_lowering=False, debug=False,
                   num_devices=N_CORES, num_swdge_queues=2)
    x = nc.dram_tensor("x", [ROWS, H], mybir.dt.float32, kind="ExternalInput")
    kidx = nc.dram_tensor("kidx", [P, max(n_tiles, 1)], mybir.dt.int32,
                          kind="ExternalInput")
    gidx = nc.dram_tensor("gidx", [P, max(n_tiles, 1) * (P // 16)],
                          mybir.dt.int16, kind="ExternalInput")
    y = nc.dram_tensor("y", [ROWS, H], mybir.dt.float32, kind="ExternalOutput")

    # The fixed prefix of example e (rows [e*S, e*S + N_FIXED)) is one
    # contiguous block of N_FIXED*H floats; view it as [128, N_FIXED*H/128]
    # so the copy is partition-mapped and spreads over all 16 SDMA engines.
    # (A direct DRAM->DRAM copy measures ~100 GB/s: no partition split.)
    FW = N_FIXED * H // P  # 3264

    def fixed_view(ap, e):
        blk = ap[e * S:e * S + N_FIXED, :]          # [204, 2048] contiguous
        return blk.rearrange("s h -> (s h)").rearrange("(p f) -> p f", p=P)

    with tile.TileContext(nc) as tc:
        with (
            tc.tile_pool(name="xp", bufs=max(n_tiles, 1)) as xp,
            tc.tile_pool(name="fp", bufs=BPC) as fp,
            tc.tile_pool(name="ip", bufs=1) as ip,
        ):
            # Fixed prefix: contiguous bulk copy through SBUF. Loads on the
            # SP HWDGE ring, stores on the ACT ring -- one-directional each,
            # both concurrent with the SWDGE indirect stream below.
            for e in range(BPC):
                ft = fp.tile([P, FW], mybir.dt.float32)
                nc.sync.dma_start(out=ft[:], in_=fixed_view(x, e))
                nc.scalar.dma_start(out=fixed_view(y, e), in_=ft[:])

            if n_tiles > 0:
                # Index loads via SWDGE: Q7 emits the 128 per-partition
                # descriptors across 16 lanes in parallel (~1-2us), vs ~10us
                # of serial HWDGE descriptor-gen on a ring we want free.
                it16 = ip.tile([P, n_tiles * (P // 16)], mybir.dt.int16)
                nc.gpsimd.dma_start(out=it16[:], in_=gidx[:])
                it = ip.tile([P, n_tiles], mybir.dt.int32)
                nc.gpsimd.dma_start(out=it[:], in_=kidx[:])
                W = P // 16
                for t in range(n_tiles):
                    xt = xp.tile([P, H], mybir.dt.float32)
                    # Gather 128 scattered rows on SWDGE queue 1; scatter
                    # them back on queue 0 -- two concurrent DMA queues.
                    nc.gpsimd.dma_gather(
                        out_ap=xt[:].rearrange("p (a h) -> p a h", a=1),
                        in_ap=x[:],
                        idxs_ap=it16[:, t * W:(t + 1) * W],
                        num_idxs=P,
                        num_idxs_reg=P,
                        elem_size=H,
                        queue_num=1,
                    )
                    nc.gpsimd.indirect_dma_start(
                        out=y[:],
                        out_offset=bass.IndirectOffsetOnAxis(
                            ap=it[:, t:t + 1], axis=0),
                        in_=xt[:],
                        in_offset=None,
                        bounds_check=ROWS - 1,
                        oob_is_err=False,
                    )
    nc.compile()
    return nc


def kernel(X, idx):
    global LAST_RESULTS
    from concourse.bass_utils import run_bass_kernel_spmd

    X = np.ascontiguousarray(np.asarray(X, dtype=np.float32))
    idx = np.asarray(idx, dtype=np.int32)

    mask = _tied_dropout_mask_host(idx)          # [B, S] float32 of {0,1}
    keep = mask.reshape(N_CORES, ROWS) > 0.5     # [8, 4096] bool
    # The fixed prefix rows (s < N_FIXED of each example) are copied by the
    # static bulk DMA; only scattered kept rows go through the index table.
    keep[:, :] &= np.tile(np.arange(S) >= N_FIXED, BPC)[None, :]

    keep_rows = [np.flatnonzero(keep[c]).astype(np.int32) for c in range(N_CORES)]
    max_keep = max(len(r) for r in keep_rows)
    n_tiles = -(-max_keep // P)                  # same static tile count per core

    in_maps = []
    for c in range(N_CORES):
        nt = max(n_tiles, 1)
        r = keep_rows[c]
        # scatter table: int32, OOB padding (skipped); tile t = column t
        flat = np.full((nt * P,), OOB_IDX, dtype=np.int32)
        flat[: len(r)] = r
        kidx = np.ascontiguousarray(flat.reshape(nt, P).T)
        # gather table: int16, slot k of tile t at [k%16, t*8 + k//16],
        # replicated down the partitions for the 8 Q7 cores. Padded with a
        # duplicate valid row (num_idxs_reg must equal the valid count);
        # the scatter's OOB padding discards the duplicates.
        gflat = np.full((nt * P,), r[-1] if len(r) else 0, dtype=np.int16)
        gflat[: len(r)] = r.astype(np.int16)
        blk = np.ascontiguousarray(gflat.reshape(nt * P // 16, 16).T)
        gidx = np.tile(blk, (P // 16, 1))
        in_maps.append({
            "x": X[c * BPC:(c + 1) * BPC].reshape(ROWS, H),
            "kidx": kidx,
            "gidx": gidx,
        })

    if n_tiles not in _PROGRAM_CACHE:
        _PROGRAM_CACHE[n_tiles] = _build_program(n_tiles)
    nc = _PROGRAM_CACHE[n_tiles]

    res = run_bass_kernel_spmd(nc, in_maps, list(range(N_CORES)))
    LAST_RESULTS = res

    out = np.empty((B, S, H), dtype=np.float32)
    for c in range(N_CORES):
        out[c * BPC:(c + 1) * BPC] = res.results[c]["y"].reshape(BPC, S, H)
    return out


# revision 17
# speedup vs baseline: 1.2530x; 1.1740x over previous
"""Trainium2 Bass kernel for nn_ExampleTiedDropout (scatter_memory).

reference: out = X * mask[:, :, None] where mask[b] has the first
int(0.2*S)=204 positions fixed to 1 and the remaining 820 positions
Bernoulli(0.1) keyed by fold_in(key(0), idx[b]).

Since the mask is exactly {0, 1}, the output is a row-sparse copy of X:
~28% of the (b, s) rows are copied verbatim, the rest are zero. The
kernel:
  1. computes the mask on host with the same jax ops as the reference
     (bit-exact: same env -> same rbg backend bitstream),
  2. shards the batch data-parallel across 8 NeuronCores (4 examples =
     4096 rows of 2048 floats per core),
  3. copies the always-kept fixed prefix (204 rows/example, contiguous,
     71% of kept bytes) as bulk partition-mapped DMAs: examples 0-2 on
     the two HWDGE rings (loads on SP, stores on ACT), example 3 on the
     SWDGE queue -- three DMA queues running concurrently,
  4. moves the ~330 scattered kept rows per core with indirect
     gather/scatter DMAs driven by a host-built row-index table (padded
     entries are out-of-bounds and skipped),
  5. leaves dropped rows untouched -- ExternalOutput buffers are donated
     zero-filled buffers, so unwritten rows read back as 0.
Raw Bass engine blocks with manual semaphores (no TileContext): Tile
inserts a false WAW dependency that serializes the scatters behind all
fixed stores; manual sems let all three queues run concurrently.
"""
import numpy as np

B, S, H = 32, 1024, 2048
N_CORES = 8
BPC = B // N_CORES           # examples per core
ROWS = BPC * S               # rows of H floats per core
P = 128                      # SBUF partitions
P_FIXED, P_MEM, MASK_SEED = 0.2, 0.1, 0
N_FIXED = int(P_FIXED * S)   # 204 leading rows per example, always kept
FW = N_FIXED * H // P        # fixed block viewed as [128, FW] (3264)
OOB_IDX = 1 << 24            # padded index; > bounds_check -> DMA skips it
NQ_EX = BPC - 1              # fixed examples on the HWDGE rings (e3 -> SWDGE)
HALF = FW // 2

_PROGRAM_CACHE = {}
LAST_RESULTS = None


def _ensure_ntff_hook():
    """The concourse trace path imports antenv.axon_hooks, which this image's
    antenv package lacks -- a hard crash when tracing is requested. Provide
    the missing module and register the boot's ctypes-based hook so NTFF
    profiling works as designed. No-op when the real module exists."""
    try:
        import antenv.axon_hooks  # noqa: F401
        return
    except ImportError:
        pass
    import sys
    import types

    mod = types.ModuleType("antenv.axon_hooks")
    mod._hook = None
    mod.set_axon_ntff_profile_hook = lambda h: setattr(mod, "_hook", h)
    mod.get_axon_ntff_profile_hook = lambda: mod._hook
    sys.modules["antenv.axon_hooks"] = mod
    try:
        import antenv
        antenv.axon_hooks = mod
    except ImportError:
        pass
    try:
        from trn_agent_boot.trn_boot import _ntff_profile_via_ctypes
        mod._hook = _ntff_profile_via_ctypes("/opt/axon/libaxon_pjrt.so")
    except Exception:
        pass  # hook stays None: concourse logs a warning and skips tracing


_ensure_ntff_hook()


def _tied_dropout_mask_host(idx_np):
    """Verbatim replica of reference._tied_dropout_mask, evaluated with the
    process-default jax backend/PRNG so the bits match the grader's
    reference run in the same environment."""
    import jax
    import jax.numpy as jnp

    n_fixed = int(P_FIXED * S)
    n_rand = S - n_fixed
    base = jax.random.key(MASK_SEED)

    def row_mask(i):
        k = jax.random.fold_in(base, i)
        return jax.random.bernoulli(k, P_MEM, (n_rand,)).astype(jnp.float32)

    idx = jnp.asarray(idx_np)
    rand_part = jax.vmap(row_mask)(idx)
    fixed_part = jnp.ones((idx.shape[0], n_fixed), jnp.float32)
    return np.asarray(jnp.concatenate([fixed_part, rand_part], axis=1))


def _fixed_view(ap, e, h):
    """Half h of example e's fixed prefix as a [128, FW/2] partition-mapped
    view of the contiguous block (rows e*S .. e*S+N_FIXED)."""
    blk = ap[e * S:e * S + N_FIXED, :]
    flat = blk.rearrange("s h -> (s h)").rearrange("(p f) -> p f", p=P)
    return flat[:, h * HALF:(h + 1) * HALF]


def _build_program(n_tiles):
    """Raw-Bass SPMD program per core. Three concurrent DMA streams:
      SP ring   : fixed loads, examples 0..2 (x -> SBUF, 2 halves each)
      ACT ring  : fixed stores, examples 0..2 (SBUF -> y, after own load)
      SWDGE q0  : idx load, example 3 fixed copy, n_tiles x (indirect
                  gather 128 rows -> SBUF, indirect scatter -> y)
    Value-agnostic: row choices live in the kidx tensor; padded entries
    are out-of-bounds and skipped by the DMA engine."""
    from contextlib import ExitStack

    import concourse.bacc as bacc
    import concourse.bass as bass
    from concourse import mybir

    nt = n_tiles
    nc = bacc.Bacc("TRN2", target_bir_lowering=False, debug=False,
                   num_devices=N_CORES)
    x = nc.dram_tensor("x", [ROWS, H], mybir.dt.float32, kind="ExternalInput")
    kidx = nc.dram_tensor("kidx", [P, max(nt, 1)], mybir.dt.int32,
                          kind="ExternalInput")
    y = nc.dram_tensor("y", [ROWS, H], mybir.dt.float32, kind="ExternalOutput")

    # Each wait below is exact: a semaphore's threshold 16*k is reached only
    # when all k DMAs that increment it have fully completed (a shared
    # counter across more DMAs could hit the threshold with partial
    # completions from later transfers).
    with ExitStack() as ctx:
        fts = ctx.enter_context(
            nc.sbuf_tensor([P, NQ_EX * FW], mybir.dt.float32))
        ft3 = ctx.enter_context(nc.sbuf_tensor([P, FW], mybir.dt.float32))
        xts = ctx.enter_context(
            nc.sbuf_tensor([P, max(nt, 1) * H], mybir.dt.float32))
        it = ctx.enter_context(
            nc.sbuf_tensor([P, max(nt, 1)], mybir.dt.int32))
        s_ld = [ctx.enter_context(nc.semaphore(f"s_ld{k}"))
                for k in range(2 * NQ_EX)]          # one per fixed half-load
        s_st = ctx.enter_context(nc.semaphore("s_st"))   # all ACT stores
        s_idx = ctx.enter_context(nc.semaphore("s_idx"))
        s_f3 = ctx.enter_context(nc.semaphore("s_f3"))   # both ft3 loads
        s_g = [ctx.enter_context(nc.semaphore(f"s_g{t}"))
               for t in range(nt)]                  # one per gather
        s_out = ctx.enter_context(nc.semaphore("s_out"))  # q0 writes
        block = ctx.enter_context(nc.Block())

        @block.sync
        def _(sync):
            for e in range(NQ_EX):
                for h in range(2):
                    sync.dma_start(
                        out=fts[:, e * FW + h * HALF:e * FW + (h + 1) * HALF],
                        in_=_fixed_view(x, e, h),
                    ).then_inc(s_ld[e * 2 + h], 16)

        @block.scalar
        def _(scalar):
            for e in range(NQ_EX):
                for h in range(2):
                    scalar.wait_ge(s_ld[e * 2 + h], 16)
                    scalar.dma_start(
                        out=_fixed_view(y, e, h),
                        in_=fts[:, e * FW + h * HALF:e * FW + (h + 1) * HALF],
                    ).then_inc(s_st, 16)
            scalar.wait_ge(s_st, 16 * 2 * NQ_EX)

        @block.gpsimd
        def _(gpsimd):
            # idx table first (gather/scatter desc-gen reads it), then
            # example 3's fixed load starts flowing while gathers emit.
            if nt > 0:
                gpsimd.dma_start(out=it[:], in_=kidx[:]).then_inc(s_idx, 16)
            e3 = NQ_EX
            gpsimd.dma_start(
                out=ft3[:, :HALF], in_=_fixed_view(x, e3, 0)
            ).then_inc(s_f3, 16)
            gpsimd.dma_start(
                out=ft3[:, HALF:], in_=_fixed_view(x, e3, 1)
            ).then_inc(s_f3, 16)
            for t in range(nt):
                if t == 0:
                    gpsimd.wait_ge(s_idx, 16)
                gpsimd.indirect_dma_start(
                    out=xts[:, t * H:(t + 1) * H],
                    out_offset=None,
                    in_=x[:],
                    in_offset=bass.IndirectOffsetOnAxis(
                        ap=it[:, t:t + 1], axis=0),
                    bounds_check=ROWS - 1,
                    oob_is_err=False,
                ).then_inc(s_g[t], 16)
            gpsimd.wait_ge(s_f3, 32)
            gpsimd.dma_start(
                out=_fixed_view(y, e3, 0), in_=ft3[:, :HALF]
            ).then_inc(s_out, 16)
            gpsimd.dma_start(
                out=_fixed_view(y, e3, 1), in_=ft3[:, HALF:]
            ).then_inc(s_out, 16)
            for t in range(nt):
                gpsimd.wait_ge(s_g[t], 16)
                gpsimd.indirect_dma_start(
                    out=y[:],
                    out_offset=bass.IndirectOffsetOnAxis(
                        ap=it[:, t:t + 1], axis=0),
                    in_=xts[:, t * H:(t + 1) * H],
                    in_offset=None,
                    bounds_check=ROWS - 1,
                    oob_is_err=False,
                ).then_inc(s_out, 16)
            gpsimd.wait_ge(s_out, 16 * (2 + nt))

    nc.compile()
    return nc


def kernel(X, idx):
    global LAST_RESULTS
    from concourse.bass_utils import run_bass_kernel_spmd

    X = np.ascontiguousarray(np.asarray(X, dtype=np.float32))
    idx = np.asarray(idx, dtype=np.int32)

    mask = _tied_dropout_mask_host(idx)          # [B, S] float32 of {0,1}
    keep = mask.reshape(N_CORES, ROWS) > 0.5     # [8, 4096] bool
    # The fixed prefix rows (s < N_FIXED of each example) are copied by the
    # static bulk DMAs; only scattered kept rows go through the index table.
    keep[:, :] &= np.tile(np.arange(S) >= N_FIXED, BPC)[None, :]

    keep_rows = [np.flatnonzero(keep[c]).astype(np.int32) for c in range(N_CORES)]
    max_keep = max(len(r) for r in keep_rows)
    n_tiles = -(-max_keep // P)                  # same static tile count per core

    in_maps = []
    for c in range(N_CORES):
        nt = max(n_tiles, 1)
        r = keep_rows[c]
        # scatter/gather table: int32, OOB padding (skipped); tile t = col t
        flat = np.full((nt * P,), OOB_IDX, dtype=np.int32)
        flat[: len(r)] = r
        kidx = np.ascontiguousarray(flat.reshape(nt, P).T)
        in_maps.append({
            "x": X[c * BPC:(c + 1) * BPC].reshape(ROWS, H),
            "kidx": kidx,
        })

    if n_tiles not in _PROGRAM_CACHE:
        _PROGRAM_CACHE[n_tiles] = _build_program(n_tiles)
    nc = _PROGRAM_CACHE[n_tiles]

    res = run_bass_kernel_spmd(nc, in_maps, list(range(N_CORES)))
    LAST_RESULTS = res

    out = np.empty((B, S, H), dtype=np.float32)
    for c in range(N_CORES):
        out[c * BPC:(c + 1) * BPC] = res.results[c]["y"].reshape(BPC, S, H)
    return out


# revision 20
# speedup vs baseline: 1.2564x; 1.0027x over previous
"""Trainium2 Bass kernel for nn_ExampleTiedDropout (scatter_memory).

reference: out = X * mask[:, :, None] where mask[b] has the first
int(0.2*S)=204 positions fixed to 1 and the remaining 820 positions
Bernoulli(0.1) keyed by fold_in(key(0), idx[b]).

Since the mask is exactly {0, 1}, the output is a row-sparse copy of X:
~28% of the (b, s) rows are copied verbatim, the rest are zero. The
kernel:
  1. computes the mask on host with the same jax ops as the reference
     (bit-exact: same env -> same rbg backend bitstream),
  2. shards the batch data-parallel across 8 NeuronCores (4 examples =
     4096 rows of 2048 floats per core),
  3. copies the always-kept fixed prefix (204 rows/example, contiguous,
     71% of kept bytes) as bulk partition-mapped DMAs: examples 0-2 on
     the two HWDGE rings (loads on SP, stores on ACT), example 3 on the
     SWDGE queue -- three DMA queues running concurrently,
  4. moves the ~330 scattered kept rows per core with indirect
     gather/scatter DMAs driven by a host-built row-index table (padded
     entries are out-of-bounds and skipped),
  5. leaves dropped rows untouched -- ExternalOutput buffers are donated
     zero-filled buffers, so unwritten rows read back as 0.
Raw Bass engine blocks with manual semaphores (no TileContext): Tile
inserts a false WAW dependency that serializes the scatters behind all
fixed stores; manual sems let all three queues run concurrently.
"""
import numpy as np

B, S, H = 32, 1024, 2048
N_CORES = 8
BPC = B // N_CORES           # examples per core
ROWS = BPC * S               # rows of H floats per core
P = 128                      # SBUF partitions
P_FIXED, P_MEM, MASK_SEED = 0.2, 0.1, 0
N_FIXED = int(P_FIXED * S)   # 204 leading rows per example, always kept
FW = N_FIXED * H // P        # fixed block viewed as [128, FW] (3264)
OOB_IDX = 1 << 24            # padded index; > bounds_check -> DMA skips it
QW = FW // 4                 # fixed blocks move in quarter chunks (816 cols)

_PROGRAM_CACHE = {}
LAST_RESULTS = None


def _ensure_ntff_hook():
    """The concourse trace path imports antenv.axon_hooks, which this image's
    antenv package lacks -- a hard crash when tracing is requested. Provide
    the missing module and register the boot's ctypes-based hook so NTFF
    profiling works as designed. No-op when the real module exists."""
    try:
        import antenv.axon_hooks  # noqa: F401
        return
    except ImportError:
        pass
    import sys
    import types

    mod = types.ModuleType("antenv.axon_hooks")
    mod._hook = None
    mod.set_axon_ntff_profile_hook = lambda h: setattr(mod, "_hook", h)
    mod.get_axon_ntff_profile_hook = lambda: mod._hook
    sys.modules["antenv.axon_hooks"] = mod
    try:
        import antenv
        antenv.axon_hooks = mod
    except ImportError:
        pass
    try:
        from trn_agent_boot.trn_boot import _ntff_profile_via_ctypes
        mod._hook = _ntff_profile_via_ctypes("/opt/axon/libaxon_pjrt.so")
    except Exception:
        pass  # hook stays None: concourse logs a warning and skips tracing


_ensure_ntff_hook()


def _tied_dropout_mask_host(idx_np):
    """Verbatim replica of reference._tied_dropout_mask, evaluated with the
    process-default jax backend/PRNG so the bits match the grader's
    reference run in the same environment."""
    import jax
    import jax.numpy as jnp

    n_fixed = int(P_FIXED * S)
    n_rand = S - n_fixed
    base = jax.random.key(MASK_SEED)

    def row_mask(i):
        k = jax.random.fold_in(base, i)
        return jax.random.bernoulli(k, P_MEM, (n_rand,)).astype(jnp.float32)

    idx = jnp.asarray(idx_np)
    rand_part = jax.vmap(row_mask)(idx)
    fixed_part = jnp.ones((idx.shape[0], n_fixed), jnp.float32)
    return np.asarray(jnp.concatenate([fixed_part, rand_part], axis=1))


def _fixed_view(ap, e, q):
    """Quarter q of example e's fixed prefix as a [128, FW/4] partition-
    mapped view of the contiguous block (rows e*S .. e*S+N_FIXED)."""
    blk = ap[e * S:e * S + N_FIXED, :]
    flat = blk.rearrange("s h -> (s h)").rearrange("(p f) -> p f", p=P)
    return flat[:, q * QW:(q + 1) * QW]


def _build_program(n_tiles):
    """Raw-Bass SPMD program per core. Three concurrent DMA streams:
      SP ring   : fixed loads, examples 0..2 (x -> SBUF, 2 halves each)
      ACT ring  : fixed stores, examples 0..2 (SBUF -> y, after own load)
      SWDGE q0  : idx load, example 3 fixed copy, n_tiles x (indirect
                  gather 128 rows -> SBUF, indirect scatter -> y)
    Value-agnostic: row choices live in the kidx tensor; padded entries
    are out-of-bounds and skipped by the DMA engine."""
    from contextlib import ExitStack

    import concourse.bacc as bacc
    import concourse.bass as bass
    from concourse import mybir

    nt = n_tiles
    nc = bacc.Bacc("TRN2", target_bir_lowering=False, debug=False,
                   num_devices=N_CORES)
    x = nc.dram_tensor("x", [ROWS, H], mybir.dt.float32, kind="ExternalInput")
    kidx = nc.dram_tensor("kidx", [P, max(nt, 1)], mybir.dt.int32,
                          kind="ExternalInput")
    y = nc.dram_tensor("y", [ROWS, H], mybir.dt.float32, kind="ExternalOutput")

    # Each wait below is exact: a semaphore's threshold 16*k is reached only
    # when all k DMAs that increment it have fully completed (a shared
    # counter across more DMAs could hit the threshold with partial
    # completions from later transfers).
    # Work split (quarter chunks of each example's fixed block):
    #   SP ring  : loads  e0-e2 (12 quarters) + e3 quarters 0-2  = 6.26 MB
    #   ACT ring : stores e0-e2 (12 quarters) + e3 quarters 0-2  = 6.26 MB
    #   SWDGE q0 : idx load, e3 quarter 3 copy, gathers+scatters = 6.24 MB
    # The three streams carry near-equal bytes so they finish together
    # (per-packet round-robin gives each queue a similar share of the
    # ~420 GB/s per-core HBM bandwidth).
    sp_loads = [(e, q) for e in range(BPC - 1) for q in range(4)]
    sp_loads += [(BPC - 1, q) for q in range(3)]
    with ExitStack() as ctx:
        fts = ctx.enter_context(
            nc.sbuf_tensor([P, BPC * FW], mybir.dt.float32))
        xts = ctx.enter_context(
            nc.sbuf_tensor([P, max(nt, 1) * H], mybir.dt.float32))
        it = ctx.enter_context(
            nc.sbuf_tensor([P, max(nt, 1)], mybir.dt.int32))
        s_ld = [ctx.enter_context(nc.semaphore(f"s_ld{k}"))
                for k in range(len(sp_loads))]      # one per SP quarter-load
        s_st = ctx.enter_context(nc.semaphore("s_st"))   # all ACT stores
        s_idx = ctx.enter_context(nc.semaphore("s_idx"))
        s_f3 = ctx.enter_context(nc.semaphore("s_f3"))   # q0 quarter load
        s_g = [ctx.enter_context(nc.semaphore(f"s_g{t}"))
               for t in range(nt)]                  # one per gather
        s_out = ctx.enter_context(nc.semaphore("s_out"))  # q0 writes
        block = ctx.enter_context(nc.Block())

        def ft_view(e, q):
            return fts[:, e * FW + q * QW:e * FW + (q + 1) * QW]

        @block.sync
        def _(sync):
            for k, (e, q) in enumerate(sp_loads):
                sync.dma_start(
                    out=ft_view(e, q), in_=_fixed_view(x, e, q)
                ).then_inc(s_ld[k], 16)

        @block.scalar
        def _(scalar):
            for k, (e, q) in enumerate(sp_loads):
                scalar.wait_ge(s_ld[k], 16)
                scalar.dma_start(
                    out=_fixed_view(y, e, q), in_=ft_view(e, q)
                ).then_inc(s_st, 16)
            scalar.wait_ge(s_st, 16 * len(sp_loads))

        @block.gpsimd
        def _(gpsimd):
            # idx table first (gather/scatter desc-gen reads it), then
            # e3's last quarter load starts flowing while gathers emit.
            e3 = BPC - 1
            if nt > 0:
                gpsimd.dma_start(out=it[:], in_=kidx[:]).then_inc(s_idx, 16)
            gpsimd.dma_start(
                out=ft_view(e3, 3), in_=_fixed_view(x, e3, 3)
            ).then_inc(s_f3, 16)
            for t in range(nt):
                if t == 0:
                    gpsimd.wait_ge(s_idx, 16)
                gpsimd.indirect_dma_start(
                    out=xts[:, t * H:(t + 1) * H],
                    out_offset=None,
                    in_=x[:],
                    in_offset=bass.IndirectOffsetOnAxis(
                        ap=it[:, t:t + 1], axis=0),
                    bounds_check=ROWS - 1,
                    oob_is_err=False,
                ).then_inc(s_g[t], 16)
            gpsimd.wait_ge(s_f3, 16)
            gpsimd.dma_start(
                out=_fixed_view(y, e3, 3), in_=ft_view(e3, 3)
            ).then_inc(s_out, 16)
            for t in range(nt):
                gpsimd.wait_ge(s_g[t], 16)
                gpsimd.indirect_dma_start(
                    out=y[:],
                    out_offset=bass.IndirectOffsetOnAxis(
                        ap=it[:, t:t + 1], axis=0),
                    in_=xts[:, t * H:(t + 1) * H],
                    in_offset=None,
                    bounds_check=ROWS - 1,
                    oob_is_err=False,
                ).then_inc(s_out, 16)
            gpsimd.wait_ge(s_out, 16 * (1 + nt))

    nc.compile()
    return nc


def kernel(X, idx):
    global LAST_RESULTS
    from concourse.bass_utils import run_bass_kernel_spmd

    X = np.ascontiguousarray(np.asarray(X, dtype=np.float32))
    idx = np.asarray(idx, dtype=np.int32)

    mask = _tied_dropout_mask_host(idx)          # [B, S] float32 of {0,1}
    keep = mask.reshape(N_CORES, ROWS) > 0.5     # [8, 4096] bool
    # The fixed prefix rows (s < N_FIXED of each example) are copied by the
    # static bulk DMAs; only scattered kept rows go through the index table.
    keep[:, :] &= np.tile(np.arange(S) >= N_FIXED, BPC)[None, :]

    keep_rows = [np.flatnonzero(keep[c]).astype(np.int32) for c in range(N_CORES)]
    max_keep = max(len(r) for r in keep_rows)
    n_tiles = -(-max_keep // P)                  # same static tile count per core

    in_maps = []
    for c in range(N_CORES):
        nt = max(n_tiles, 1)
        r = keep_rows[c]
        # scatter/gather table: int32, OOB padding (skipped); tile t = col t
        flat = np.full((nt * P,), OOB_IDX, dtype=np.int32)
        flat[: len(r)] = r
        kidx = np.ascontiguousarray(flat.reshape(nt, P).T)
        in_maps.append({
            "x": X[c * BPC:(c + 1) * BPC].reshape(ROWS, H),
            "kidx": kidx,
        })

    if n_tiles not in _PROGRAM_CACHE:
        _PROGRAM_CACHE[n_tiles] = _build_program(n_tiles)
    nc = _PROGRAM_CACHE[n_tiles]

    res = run_bass_kernel_spmd(nc, in_maps, list(range(N_CORES)))
    LAST_RESULTS = res

    out = np.empty((B, S, H), dtype=np.float32)
    for c in range(N_CORES):
        out[c * BPC:(c + 1) * BPC] = res.results[c]["y"].reshape(BPC, S, H)
    return out


# revision 21
# speedup vs baseline: 1.2656x; 1.0073x over previous
"""Trainium2 Bass kernel for nn_ExampleTiedDropout (scatter_memory).

reference: out = X * mask[:, :, None] where mask[b] has the first
int(0.2*S)=204 positions fixed to 1 and the remaining 820 positions
Bernoulli(0.1) keyed by fold_in(key(0), idx[b]).

Since the mask is exactly {0, 1}, the output is a row-sparse copy of X:
~28% of the (b, s) rows are copied verbatim, the rest are zero. The
kernel:
  1. computes the mask on host with the same jax ops as the reference
     (bit-exact: same env -> same rbg backend bitstream),
  2. shards the batch data-parallel across 8 NeuronCores (4 examples =
     4096 rows of 2048 floats per core),
  3. copies the always-kept fixed prefix (204 rows/example, contiguous,
     71% of kept bytes) as bulk partition-mapped DMAs: examples 0-2 on
     the two HWDGE rings (loads on SP, stores on ACT), example 3 on the
     SWDGE queue -- three DMA queues running concurrently,
  4. moves the ~330 scattered kept rows per core with indirect
     gather/scatter DMAs driven by a host-built row-index table (padded
     entries are out-of-bounds and skipped),
  5. leaves dropped rows untouched -- ExternalOutput buffers are donated
     zero-filled buffers, so unwritten rows read back as 0.
Raw Bass engine blocks with manual semaphores (no TileContext): Tile
inserts a false WAW dependency that serializes the scatters behind all
fixed stores; manual sems let all three queues run concurrently.
"""
import numpy as np

B, S, H = 32, 1024, 2048
N_CORES = 8
BPC = B // N_CORES           # examples per core
ROWS = BPC * S               # rows of H floats per core
P = 128                      # SBUF partitions
P_FIXED, P_MEM, MASK_SEED = 0.2, 0.1, 0
N_FIXED = int(P_FIXED * S)   # 204 leading rows per example, always kept
FW = N_FIXED * H // P        # fixed block viewed as [128, FW] (3264)
OOB_IDX = 1 << 24            # padded index; > bounds_check -> DMA skips it
QW = FW // 4                 # fixed blocks move in quarter chunks (816 cols)

_PROGRAM_CACHE = {}
LAST_RESULTS = None


def _ensure_ntff_hook():
    """The concourse trace path imports antenv.axon_hooks, which this image's
    antenv package lacks -- a hard crash when tracing is requested. Provide
    the missing module and register the boot's ctypes-based hook so NTFF
    profiling works as designed. No-op when the real module exists."""
    try:
        import antenv.axon_hooks  # noqa: F401
        return
    except ImportError:
        pass
    import sys
    import types

    mod = types.ModuleType("antenv.axon_hooks")
    mod._hook = None
    mod.set_axon_ntff_profile_hook = lambda h: setattr(mod, "_hook", h)
    mod.get_axon_ntff_profile_hook = lambda: mod._hook
    sys.modules["antenv.axon_hooks"] = mod
    try:
        import antenv
        antenv.axon_hooks = mod
    except ImportError:
        pass
    try:
        from trn_agent_boot.trn_boot import _ntff_profile_via_ctypes
        mod._hook = _ntff_profile_via_ctypes("/opt/axon/libaxon_pjrt.so")
    except Exception:
        pass  # hook stays None: concourse logs a warning and skips tracing


_ensure_ntff_hook()


def _tied_dropout_mask_host(idx_np):
    """Verbatim replica of reference._tied_dropout_mask, evaluated with the
    process-default jax backend/PRNG so the bits match the grader's
    reference run in the same environment."""
    import jax
    import jax.numpy as jnp

    n_fixed = int(P_FIXED * S)
    n_rand = S - n_fixed
    base = jax.random.key(MASK_SEED)

    def row_mask(i):
        k = jax.random.fold_in(base, i)
        return jax.random.bernoulli(k, P_MEM, (n_rand,)).astype(jnp.float32)

    idx = jnp.asarray(idx_np)
    rand_part = jax.vmap(row_mask)(idx)
    fixed_part = jnp.ones((idx.shape[0], n_fixed), jnp.float32)
    return np.asarray(jnp.concatenate([fixed_part, rand_part], axis=1))


def _fixed_view(ap, e, q):
    """Quarter q of example e's fixed prefix as a [128, FW/4] partition-
    mapped view of the contiguous block (rows e*S .. e*S+N_FIXED)."""
    blk = ap[e * S:e * S + N_FIXED, :]
    flat = blk.rearrange("s h -> (s h)").rearrange("(p f) -> p f", p=P)
    return flat[:, q * QW:(q + 1) * QW]


def _build_program(n_tiles):
    """Raw-Bass SPMD program per core. Three concurrent DMA streams:
      SP ring   : fixed loads, examples 0..2 (x -> SBUF, 2 halves each)
      ACT ring  : fixed stores, examples 0..2 (SBUF -> y, after own load)
      SWDGE q0  : idx load, example 3 fixed copy, n_tiles x (indirect
                  gather 128 rows -> SBUF, indirect scatter -> y)
    Value-agnostic: row choices live in the kidx tensor; padded entries
    are out-of-bounds and skipped by the DMA engine."""
    from contextlib import ExitStack

    import concourse.bacc as bacc
    import concourse.bass as bass
    from concourse import mybir

    nt = n_tiles
    nc = bacc.Bacc("TRN2", target_bir_lowering=False, debug=False,
                   num_devices=N_CORES)
    x = nc.dram_tensor("x", [ROWS, H], mybir.dt.float32, kind="ExternalInput")
    kidx = nc.dram_tensor("kidx", [P, max(nt, 1)], mybir.dt.int32,
                          kind="ExternalInput")
    y = nc.dram_tensor("y", [ROWS, H], mybir.dt.float32, kind="ExternalOutput")

    # Each wait below is exact: a semaphore's threshold 16*k is reached only
    # when all k DMAs that increment it have fully completed (a shared
    # counter across more DMAs could hit the threshold with partial
    # completions from later transfers).
    # Work split (quarter chunks of each example's fixed block):
    #   SP ring  : loads  e0-e2 (12 quarters) + e3 quarters 0-2  = 6.26 MB
    #   ACT ring : stores e0-e2 (12 quarters) + e3 quarters 0-2  = 6.26 MB
    #   SWDGE q0 : idx load, e3 quarter 3 copy, gathers+scatters = 6.24 MB
    # The three streams carry near-equal bytes so they finish together
    # (per-packet round-robin gives each queue a similar share of the
    # ~420 GB/s per-core HBM bandwidth).
    sp_loads = [(e, q) for e in range(BPC - 1) for q in range(4)]
    sp_loads += [(BPC - 1, q) for q in range(3)]
    with ExitStack() as ctx:
        fts = ctx.enter_context(
            nc.sbuf_tensor([P, BPC * FW], mybir.dt.float32))
        xts = ctx.enter_context(
            nc.sbuf_tensor([P, max(nt, 1) * H], mybir.dt.float32))
        it = ctx.enter_context(
            nc.sbuf_tensor([P, max(nt, 1)], mybir.dt.int32))
        s_ld = [ctx.enter_context(nc.semaphore(f"s_ld{k}"))
                for k in range(len(sp_loads))]      # one per SP quarter-load
        s_st = ctx.enter_context(nc.semaphore("s_st"))   # all ACT stores
        s_idx = ctx.enter_context(nc.semaphore("s_idx"))
        s_f3 = ctx.enter_context(nc.semaphore("s_f3"))   # q0 quarter load
        s_g = [ctx.enter_context(nc.semaphore(f"s_g{t}"))
               for t in range(nt)]                  # one per gather
        s_out = ctx.enter_context(nc.semaphore("s_out"))  # q0 writes
        # All DMA completions are already enforced by the explicit semaphore
        # waits above, so skip GpSimd's expensive DGE drain at block exit and
        # use the cheaper sem-only all-engine barrier.
        block = ctx.enter_context(nc.Block(no_gpsimd_drain=True))

        def ft_view(e, q):
            return fts[:, e * FW + q * QW:e * FW + (q + 1) * QW]

        @block.sync
        def _(sync):
            for k, (e, q) in enumerate(sp_loads):
                sync.dma_start(
                    out=ft_view(e, q), in_=_fixed_view(x, e, q)
                ).then_inc(s_ld[k], 16)

        @block.scalar
        def _(scalar):
            for k, (e, q) in enumerate(sp_loads):
                scalar.wait_ge(s_ld[k], 16)
                scalar.dma_start(
                    out=_fixed_view(y, e, q), in_=ft_view(e, q)
                ).then_inc(s_st, 16)
            scalar.wait_ge(s_st, 16 * len(sp_loads))

        @block.gpsimd
        def _(gpsimd):
            # idx table first (gather/scatter desc-gen reads it), then
            # e3's last quarter load starts flowing while gathers emit.
            e3 = BPC - 1
            if nt > 0:
                gpsimd.dma_start(out=it[:], in_=kidx[:]).then_inc(s_idx, 16)
            gpsimd.dma_start(
                out=ft_view(e3, 3), in_=_fixed_view(x, e3, 3)
            ).then_inc(s_f3, 16)
            for t in range(nt):
                if t == 0:
                    gpsimd.wait_ge(s_idx, 16)
                gpsimd.indirect_dma_start(
                    out=xts[:, t * H:(t + 1) * H],
                    out_offset=None,
                    in_=x[:],
                    in_offset=bass.IndirectOffsetOnAxis(
                        ap=it[:, t:t + 1], axis=0),
                    bounds_check=ROWS - 1,
                    oob_is_err=False,
                ).then_inc(s_g[t], 16)
            gpsimd.wait_ge(s_f3, 16)
            gpsimd.dma_start(
                out=_fixed_view(y, e3, 3), in_=ft_view(e3, 3)
            ).then_inc(s_out, 16)
            for t in range(nt):
                gpsimd.wait_ge(s_g[t], 16)
                gpsimd.indirect_dma_start(
                    out=y[:],
                    out_offset=bass.IndirectOffsetOnAxis(
                        ap=it[:, t:t + 1], axis=0),
                    in_=xts[:, t * H:(t + 1) * H],
                    in_offset=None,
                    bounds_check=ROWS - 1,
                    oob_is_err=False,
                ).then_inc(s_out, 16)
            gpsimd.wait_ge(s_out, 16 * (1 + nt))

    nc.compile()
    return nc


def kernel(X, idx):
    global LAST_RESULTS
    from concourse.bass_utils import run_bass_kernel_spmd

    X = np.ascontiguousarray(np.asarray(X, dtype=np.float32))
    idx = np.asarray(idx, dtype=np.int32)

    mask = _tied_dropout_mask_host(idx)          # [B, S] float32 of {0,1}
    keep = mask.reshape(N_CORES, ROWS) > 0.5     # [8, 4096] bool
    # The fixed prefix rows (s < N_FIXED of each example) are copied by the
    # static bulk DMAs; only scattered kept rows go through the index table.
    keep[:, :] &= np.tile(np.arange(S) >= N_FIXED, BPC)[None, :]

    keep_rows = [np.flatnonzero(keep[c]).astype(np.int32) for c in range(N_CORES)]
    max_keep = max(len(r) for r in keep_rows)
    n_tiles = -(-max_keep // P)                  # same static tile count per core

    in_maps = []
    for c in range(N_CORES):
        nt = max(n_tiles, 1)
        r = keep_rows[c]
        # scatter/gather table: int32, OOB padding (skipped); tile t = col t
        flat = np.full((nt * P,), OOB_IDX, dtype=np.int32)
        flat[: len(r)] = r
        kidx = np.ascontiguousarray(flat.reshape(nt, P).T)
        in_maps.append({
            "x": X[c * BPC:(c + 1) * BPC].reshape(ROWS, H),
            "kidx": kidx,
        })

    if n_tiles not in _PROGRAM_CACHE:
        _PROGRAM_CACHE[n_tiles] = _build_program(n_tiles)
    nc = _PROGRAM_CACHE[n_tiles]

    res = run_bass_kernel_spmd(nc, in_maps, list(range(N_CORES)))
    LAST_RESULTS = res

    out = np.empty((B, S, H), dtype=np.float32)
    for c in range(N_CORES):
        out[c * BPC:(c + 1) * BPC] = res.results[c]["y"].reshape(BPC, S, H)
    return out


# revision 22
# speedup vs baseline: 1.7349x; 1.3708x over previous
"""Trainium2 Bass kernel for nn_ExampleTiedDropout (scatter_memory).

reference: out = X * mask[:, :, None] where mask[b] has the first
int(0.2*S)=204 positions fixed to 1 and the remaining 820 positions
Bernoulli(0.1) keyed by fold_in(key(0), idx[b]).

Since the mask is exactly {0, 1}, the output is a row-sparse copy of X:
~28% of the (b, s) rows are copied verbatim, the rest are zero. The
kernel:
  1. computes the mask on host with the same jax ops as the reference
     (bit-exact: same env -> same rbg backend bitstream),
  2. shards the batch data-parallel across 8 NeuronCores (4 examples =
     4096 rows of 2048 floats per core),
  3. copies the always-kept fixed prefix (204 rows/example, contiguous,
     71% of kept bytes) as bulk partition-mapped DMAs: examples 0-2 on
     the two HWDGE rings (loads on SP, stores on ACT), example 3 on the
     SWDGE queue -- three DMA queues running concurrently,
  4. moves the ~330 scattered kept rows per core with indirect
     gather/scatter DMAs driven by a host-built row-index table (padded
     entries are out-of-bounds and skipped),
  5. leaves dropped rows untouched -- ExternalOutput buffers are donated
     zero-filled buffers, so unwritten rows read back as 0.
Raw Bass engine blocks with manual semaphores (no TileContext): Tile
inserts a false WAW dependency that serializes the scatters behind all
fixed stores; manual sems let all three queues run concurrently.
"""
import numpy as np

B, S, H = 32, 1024, 2048
N_CORES = 8
BPC = B // N_CORES           # examples per core
ROWS = BPC * S               # rows of H floats per core
P = 128                      # SBUF partitions
P_FIXED, P_MEM, MASK_SEED = 0.2, 0.1, 0
N_FIXED = int(P_FIXED * S)   # 204 leading rows per example, always kept
FW = N_FIXED * H // P        # fixed block viewed as [128, FW] (3264)
OOB_IDX = 1 << 24            # padded index; > bounds_check -> DMA skips it
QW = FW // 4                 # fixed blocks move in quarter chunks (816 cols)

_PROGRAM_CACHE = {}
LAST_RESULTS = None


def _ensure_ntff_hook():
    """The concourse trace path imports antenv.axon_hooks, which this image's
    antenv package lacks -- a hard crash when tracing is requested. Provide
    the missing module and register the boot's ctypes-based hook so NTFF
    profiling works as designed. No-op when the real module exists."""
    try:
        import antenv.axon_hooks  # noqa: F401
        return
    except ImportError:
        pass
    import sys
    import types

    mod = types.ModuleType("antenv.axon_hooks")
    mod._hook = None
    mod.set_axon_ntff_profile_hook = lambda h: setattr(mod, "_hook", h)
    mod.get_axon_ntff_profile_hook = lambda: mod._hook
    sys.modules["antenv.axon_hooks"] = mod
    try:
        import antenv
        antenv.axon_hooks = mod
    except ImportError:
        pass
    try:
        from trn_agent_boot.trn_boot import _ntff_profile_via_ctypes
        mod._hook = _ntff_profile_via_ctypes("/opt/axon/libaxon_pjrt.so")
    except Exception:
        pass  # hook stays None: concourse logs a warning and skips tracing


_ensure_ntff_hook()


def _tied_dropout_mask_host(idx_np):
    """Verbatim replica of reference._tied_dropout_mask, evaluated with the
    process-default jax backend/PRNG so the bits match the grader's
    reference run in the same environment."""
    import jax
    import jax.numpy as jnp

    n_fixed = int(P_FIXED * S)
    n_rand = S - n_fixed
    base = jax.random.key(MASK_SEED)

    def row_mask(i):
        k = jax.random.fold_in(base, i)
        return jax.random.bernoulli(k, P_MEM, (n_rand,)).astype(jnp.float32)

    idx = jnp.asarray(idx_np)
    rand_part = jax.vmap(row_mask)(idx)
    fixed_part = jnp.ones((idx.shape[0], n_fixed), jnp.float32)
    return np.asarray(jnp.concatenate([fixed_part, rand_part], axis=1))


def _fixed_view(ap, e, q):
    """Quarter q of example e's fixed prefix as a [128, FW/4] partition-
    mapped view of the contiguous block (rows e*S .. e*S+N_FIXED)."""
    blk = ap[e * S:e * S + N_FIXED, :]
    flat = blk.rearrange("s h -> (s h)").rearrange("(p f) -> p f", p=P)
    return flat[:, q * QW:(q + 1) * QW]


def _build_program(n_tiles):
    """Raw-Bass SPMD program per core. Three concurrent DMA streams:
      SP ring   : fixed loads, examples 0..2 (x -> SBUF, 2 halves each)
      ACT ring  : fixed stores, examples 0..2 (SBUF -> y, after own load)
      SWDGE q0  : idx load, example 3 fixed copy, n_tiles x (indirect
                  gather 128 rows -> SBUF, indirect scatter -> y)
    Value-agnostic: row choices live in the kidx tensor; padded entries
    are out-of-bounds and skipped by the DMA engine."""
    from contextlib import ExitStack

    import concourse.bacc as bacc
    import concourse.bass as bass
    from concourse import mybir

    nt = n_tiles
    nc = bacc.Bacc("TRN2", target_bir_lowering=False, debug=False,
                   num_devices=N_CORES)
    x = nc.dram_tensor("x", [ROWS, H], mybir.dt.float32, kind="ExternalInput")
    kidx = nc.dram_tensor("kidx", [P, max(nt, 1)], mybir.dt.int32,
                          kind="ExternalInput")
    y = nc.dram_tensor("y", [ROWS, H], mybir.dt.float32, kind="ExternalOutput")

    # Each wait below is exact: a semaphore's threshold 16*k is reached only
    # when all k DMAs that increment it have fully completed (a shared
    # counter across more DMAs could hit the threshold with partial
    # completions from later transfers).
    # Work split: fixed blocks move as direct DRAM->DRAM copies (no SBUF
    # bounce, no load->store chain): examples 0-1 on the SP ring, 2-3 on
    # the ACT ring; SWDGE q0 carries the idx load + indirect gathers and
    # scatters of the scattered kept rows.
    with ExitStack() as ctx:
        xts = ctx.enter_context(
            nc.sbuf_tensor([P, max(nt, 1) * H], mybir.dt.float32))
        it = ctx.enter_context(
            nc.sbuf_tensor([P, max(nt, 1)], mybir.dt.int32))
        s_cp = ctx.enter_context(nc.semaphore("s_cp"))    # ring D2D copies
        s_idx = ctx.enter_context(nc.semaphore("s_idx"))
        s_g = [ctx.enter_context(nc.semaphore(f"s_g{t}"))
               for t in range(nt)]                  # one per gather
        s_out = ctx.enter_context(nc.semaphore("s_out"))  # q0 scatters
        block = ctx.enter_context(nc.Block(no_gpsimd_drain=True))

        def fixed_all(ap, e):
            blk = ap[e * S:e * S + N_FIXED, :]
            return blk.rearrange("s h -> (s h)").rearrange("(p f) -> p f", p=P)

        @block.sync
        def _(sync):
            for e in (0, 1):
                for q in range(4):
                    sync.dma_start(
                        out=_fixed_view(y, e, q), in_=_fixed_view(x, e, q)
                    ).then_inc(s_cp, 16)
            sync.wait_ge(s_cp, 16 * 8)

        @block.scalar
        def _(scalar):
            for e in (2, 3):
                for q in range(4):
                    scalar.dma_start(
                        out=_fixed_view(y, e, q), in_=_fixed_view(x, e, q)
                    ).then_inc(s_cp, 16)

        @block.gpsimd
        def _(gpsimd):
            if nt > 0:
                gpsimd.dma_start(out=it[:], in_=kidx[:]).then_inc(s_idx, 16)
            for t in range(nt):
                if t == 0:
                    gpsimd.wait_ge(s_idx, 16)
                gpsimd.indirect_dma_start(
                    out=xts[:, t * H:(t + 1) * H],
                    out_offset=None,
                    in_=x[:],
                    in_offset=bass.IndirectOffsetOnAxis(
                        ap=it[:, t:t + 1], axis=0),
                    bounds_check=ROWS - 1,
                    oob_is_err=False,
                ).then_inc(s_g[t], 16)
            for t in range(nt):
                gpsimd.wait_ge(s_g[t], 16)
                gpsimd.indirect_dma_start(
                    out=y[:],
                    out_offset=bass.IndirectOffsetOnAxis(
                        ap=it[:, t:t + 1], axis=0),
                    in_=xts[:, t * H:(t + 1) * H],
                    in_offset=None,
                    bounds_check=ROWS - 1,
                    oob_is_err=False,
                ).then_inc(s_out, 16)
            gpsimd.wait_ge(s_out, 16 * nt)

    nc.compile()
    return nc


def kernel(X, idx):
    global LAST_RESULTS
    from concourse.bass_utils import run_bass_kernel_spmd

    X = np.ascontiguousarray(np.asarray(X, dtype=np.float32))
    idx = np.asarray(idx, dtype=np.int32)

    mask = _tied_dropout_mask_host(idx)          # [B, S] float32 of {0,1}
    keep = mask.reshape(N_CORES, ROWS) > 0.5     # [8, 4096] bool
    # The fixed prefix rows (s < N_FIXED of each example) are copied by the
    # static bulk DMAs; only scattered kept rows go through the index table.
    keep[:, :] &= np.tile(np.arange(S) >= N_FIXED, BPC)[None, :]

    keep_rows = [np.flatnonzero(keep[c]).astype(np.int32) for c in range(N_CORES)]
    max_keep = max(len(r) for r in keep_rows)
    n_tiles = -(-max_keep // P)                  # same static tile count per core

    in_maps = []
    for c in range(N_CORES):
        nt = max(n_tiles, 1)
        r = keep_rows[c]
        # scatter/gather table: int32, OOB padding (skipped); tile t = col t
        flat = np.full((nt * P,), OOB_IDX, dtype=np.int32)
        flat[: len(r)] = r
        kidx = np.ascontiguousarray(flat.reshape(nt, P).T)
        in_maps.append({
            "x": X[c * BPC:(c + 1) * BPC].reshape(ROWS, H),
            "kidx": kidx,
        })

    if n_tiles not in _PROGRAM_CACHE:
        _PROGRAM_CACHE[n_tiles] = _build_program(n_tiles)
    nc = _PROGRAM_CACHE[n_tiles]

    res = run_bass_kernel_spmd(nc, in_maps, list(range(N_CORES)))
    LAST_RESULTS = res

    out = np.empty((B, S, H), dtype=np.float32)
    for c in range(N_CORES):
        out[c * BPC:(c + 1) * BPC] = res.results[c]["y"].reshape(BPC, S, H)
    return out
